# revision 1
# baseline (speedup 1.0000x reference)
"""Trainium2 Bass kernel for GNN message-passing conv layer.

Reference computation:
    xs = x * symm_norm[:, None]            # [N, C]
    g  = xs[domains]                        # [D, K, C]
    f  = concat([g, g], -1)                 # [D, K, 2C]
    y  = f @ w + b                          # [D, K, CO]

Algebraic rewrites used:
    concat([g, g]) @ w == g @ (w[:C] + w[C:])       (fold doubled channels)
    (s*x) @ w == s * (x @ w)                          (scale moves post-GEMM,
                                                       fused into the PSUM drain)

Sharding: D axis data-parallel across 8 cores (3125 domains -> 50000 gathered
rows per core); x/w/b replicated. Host does marshalling only: builds a 1280B-row
gather table [x | symm_norm | pad] (256B-multiple rows for dma_gather), converts
indices to int16 with an A/B split (dma_gather indices are signed int16, so rows
>= 32768 are gathered from a base offset of 32768 with idx-32768; positions are
host-permuted so every 1024-row chunk is pure A or pure B, and the output is
unpermuted on the host), and wraps indices in the 16-partition layout the Q7
gather ucode expects.

Per-core device pipeline, per 1024-row chunk (8 row-tiles of 128):
    1x dma_gather      -> gx [128, 8, 320] f32       (gpsimd SWDGE, one instr)
    per pair of tiles: 4x PE transpose (f32) into one PSUM bank,
                       1x DVE copy [128,512] PSUM->SBUF (casts to f32r)
    per tile:          2x accumulating f32r matmuls (w_eff chunks)
                       drain = tensor_scalar mult by gathered symm_norm
                               (alternating DVE / ACT to balance engines)
    1x batched store of the chunk [1024, 256] (HWDGE)
"""

import numpy as np
from contextlib import ExitStack

import concourse.bass as bass
import concourse.bacc as bacc
import concourse.mybir as mybir
import concourse.tile as tile
from concourse.bass_utils import run_bass_kernel_spmd
from concourse.masks import make_identity

# Problem shapes (hardcoded per contract)
N, C, D, K, CO = 50000, 256, 25000, 16, 256
NCORES = 8
DPC = D // NCORES          # domains per core
RPC = DPC * K              # gathered rows per core (50000)
P = 128
EL = 320                   # gather-table row: 256 x + 1 symm_norm + 63 pad
HALF = 32768               # int16 index limit; B-region gathers from base+HALF
CHUNK = 1024               # rows per dma_gather (8 row-tiles)
TPC = CHUNK // P           # tiles per chunk (8)

# Module-level switches (test.py pokes these; harness uses defaults)
TRACE = False
TMPDIR = None

_cache = {}


def _build_nc(nac, nbc, use_f32r=True):
    """nac/nbc: number of A-region / B-region chunks."""
    f32 = mybir.dt.float32
    mmdt = mybir.dt.float32r if use_f32r else f32
    nchunks = nac + nbc
    ntp = nchunks * CHUNK

    # 4 SWDGE queues: the Q7 descriptor-emission (~8.5ns/descriptor) is the
    # serial cost of the gathers; round-robin queues parallelize it.
    nc = bacc.Bacc(num_swdge_queues=4)
    xg = nc.dram_tensor("xg", [N, EL], f32, kind="ExternalInput")
    idx = nc.dram_tensor("idx", [P, ntp // 16], mybir.dt.int16,
                         kind="ExternalInput")
    wd = nc.dram_tensor("w", [2 * C, CO], f32, kind="ExternalInput")
    out = nc.dram_tensor("out", [ntp, CO], f32, kind="ExternalOutput")

    with tile.TileContext(nc) as tc, ExitStack() as ctx:
        const = ctx.enter_context(tc.tile_pool(name="const", bufs=1))
        gxp = ctx.enter_context(tc.tile_pool(name="gx", bufs=8))
        xtp = ctx.enter_context(tc.tile_pool(name="xt", bufs=4))
        obp = ctx.enter_context(tc.tile_pool(name="ob", bufs=4))
        tpp = ctx.enter_context(tc.tile_pool(name="tp", bufs=3, space="PSUM"))
        opp = ctx.enter_context(tc.tile_pool(name="op", bufs=4, space="PSUM"))

        # --- one-time setup ---
        idx_sb = const.tile([P, ntp // 16], mybir.dt.int16)
        nc.sync.dma_start(idx_sb[:], idx[:])

        # w: [512, CO] -> [128, 4, CO] (partition p, chunk q = row q*128+p)
        wt = const.tile([P, 4, CO], f32)
        nc.sync.dma_start(wt[:], wd.rearrange("(q p) n -> p q n", p=P))
        # fold: w_eff chunk k = w[k*128:+128] + w[256 + k*128:+128]
        # (DVE output-casts to f32r when used: matmul operands must be rounded)
        we = const.tile([P, 2, CO], mmdt)
        nc.vector.tensor_add(we[:, 0, :], wt[:, 0, :], wt[:, 2, :])
        nc.vector.tensor_add(we[:, 1, :], wt[:, 1, :], wt[:, 3, :])

        ident = const.tile([P, P], f32)
        make_identity(nc, ident[:])

        # --- main loop ---
        for ci in range(nchunks):
            base = xg[:] if ci < nac else xg[HALF:, :]
            gx = gxp.tile([P, TPC, EL], f32)
            nc.gpsimd.dma_gather(
                gx[:], base, idx_sb[:, ci * (CHUNK // 16):(ci + 1) * (CHUNK // 16)],
                CHUNK, CHUNK, EL, queue_num=ci % 4, single_packet=False,
            )
            ob = obp.tile([P, TPC, CO], f32)
            for j2 in range(TPC // 2):
                # two row-tiles' transposes fill one PSUM bank, drained by
                # a single [128, 512] copy (cast to matmul dtype)
                tpX = tpp.tile([P, 4, P], f32)
                for jj in range(2):
                    j = 2 * j2 + jj
                    nc.tensor.transpose(tpX[:, 2 * jj + 0, :],
                                        gx[:, j, 0:P], ident[:])
                    nc.tensor.transpose(tpX[:, 2 * jj + 1, :],
                                        gx[:, j, P:C], ident[:])
                xt = xtp.tile([P, 4, P], mmdt)
                nc.vector.tensor_copy(xt[:], tpX[:])
                for jj in range(2):
                    j = 2 * j2 + jj
                    op = opp.tile([P, CO], f32)
                    nc.tensor.matmul(op[:], xt[:, 2 * jj + 0, :], we[:, 0, :],
                                     start=True, stop=False)
                    nc.tensor.matmul(op[:], xt[:, 2 * jj + 1, :], we[:, 1, :],
                                     start=False, stop=True)
                    # drain with fused symm_norm scale: y = s * (g @ w_eff)
                    # (b == 0 for this problem; a nonzero b would add a
                    # broadcast tensor_tensor add here)
                    sc = gx[:, j, C:C + 1]
                    if j % 2 == 0:
                        nc.vector.tensor_scalar_mul(ob[:, j, :], op[:], sc)
                    else:
                        nc.scalar.activation(
                            ob[:, j, :], op[:],
                            mybir.ActivationFunctionType.Copy, scale=sc)
            # one batched store per chunk: DRAM rows ci*CHUNK + j*128 + p
            nc.sync.dma_start(
                out[ci * CHUNK:(ci + 1) * CHUNK, :]
                .rearrange("(j p) n -> p j n", p=P),
                ob[:],
            )

    nc.finalize()
    return nc


def kernel(x, symm_norm, domains, w, b):
    x = np.asarray(x, dtype=np.float32)
    symm_norm = np.asarray(symm_norm, dtype=np.float32)
    domains = np.asarray(domains)
    w = np.asarray(w, dtype=np.float32)
    b = np.asarray(b, dtype=np.float32)
    assert np.all(b == 0.0), "kernel built for b == 0 (reference uses zeros)"

    # gather table [x | symm_norm | pad] with 1280B rows (marshalling only)
    xg = np.zeros((N, EL), dtype=np.float32)
    xg[:, :C] = x
    xg[:, C] = symm_norm

    # Dedup: equal indices produce identical output rows (same x row, same
    # symm_norm), so the device computes each unique row once and the host
    # unshard step fans the results back out (exact, pure result movement).
    # np.unique returns SORTED uniques: the int16 A/B split is a clean
    # prefix/suffix, and the gather pattern becomes ascending in HBM.
    dom = domains.reshape(D, K).astype(np.int64)
    cores = []
    for c in range(NCORES):
        flat = dom[c * DPC:(c + 1) * DPC].reshape(-1)
        uniq, inv = np.unique(flat, return_inverse=True)
        nA = int((uniq < HALF).sum())
        cores.append((uniq, inv, nA))

    nac = max(-(-co[2] // CHUNK) for co in cores)
    nbc = max(-(-(len(co[0]) - co[2]) // CHUNK) for co in cores)
    ntp = (nac + nbc) * CHUNK

    in_maps = []
    for uniq, inv, nA in cores:
        nB = len(uniq) - nA
        vals = np.zeros(ntp, dtype=np.int16)
        vals[:nA] = uniq[:nA]
        vals[nac * CHUNK:nac * CHUNK + nB] = uniq[nA:] - HALF
        # 16-partition wrap, replicated across the 8 Q7 cores
        v16 = vals.reshape(ntp // 16, 16).T          # [16, ntp//16]
        idx16 = np.ascontiguousarray(np.tile(v16, (8, 1)))  # [128, ntp//16]
        in_maps.append({"xg": xg, "idx": idx16, "w": w})

    key = (nac, nbc)
    if _cache.get("key") != key:
        _cache["nc"] = _build_nc(nac, nbc)
        _cache["key"] = key
    nc = _cache["nc"]

    res = run_bass_kernel_spmd(
        nc, in_maps, core_ids=list(range(NCORES)),
        trace=TRACE, tmpdir=TMPDIR,
    )
    _cache["last_results"] = res

    outs = []
    for (uniq, inv, nA), r in zip(cores, res.results):
        dev = r["out"]
        nB = len(uniq) - nA
        # unique-row results in uniq order: A-region prefix + B-region
        yu = np.concatenate(
            [dev[:nA], dev[nac * CHUNK:nac * CHUNK + nB]], axis=0)
        outs.append(yu[inv].reshape(DPC, K, CO))
    return np.concatenate(outs, axis=0)



# revision 2
# speedup vs baseline: 6.4853x; 6.4853x over previous
"""Trainium2 Bass kernel for GNN message-passing conv layer.

Reference computation:
    xs = x * symm_norm[:, None]            # [N, C]
    g  = xs[domains]                        # [D, K, C]
    f  = concat([g, g], -1)                 # [D, K, 2C]
    y  = f @ w + b                          # [D, K, CO]

Algebraic rewrites:
    concat([g, g]) @ w == g @ (w[:C] + w[C:])      (fold doubled channels)
    gather commutes with the row-wise linear map:
        y[d,k] = z[domains[d,k]]  where  z = (x * s) @ w_eff
    (s*x) @ w == s * (x @ w)                       (scale fused into PSUM drain)

So the device computes z for all N nodes once -- a dense [N, C] @ [C, CO]
GEMM row-sharded across the 8 cores (6400 rows each, zero-padded from
50000) -- and the host fans results out with z[domains] (pure result
movement, the same unshard step the gather formulation needs).

Per-core device pipeline (NCH chunks x TPCH row-tiles of 128):
    in-DMA  (SP ring):  xT chunk [128, 2*CHW] (host-marshalled transposed
                        layout: element [p, q*CHW+m] = x[row m, chan q*128+p])
    per tile: 2 accumulating matmuls (k-chunks of 128) into a PSUM tile,
              drain = scale by symm_norm[row] (alternating DVE / ACT)
    out-DMA (ACT ring): z chunk [128, TPCH*256]
w fold + cast runs once on device; symm_norm arrives pre-tiled [128, NT].
"""

import numpy as np
import ml_dtypes
from contextlib import ExitStack

import concourse.bass as bass
import concourse.bacc as bacc
import concourse.mybir as mybir
import concourse.tile as tile
from concourse.bass_utils import run_bass_kernel_spmd

# Problem shapes (hardcoded per contract)
N, C, D, K, CO = 50000, 256, 25000, 16, 256
NCORES = 8
P = 128
MPC = 6400                 # rows per core (8*6400 = 51200 >= N, zero-padded)
NT = MPC // P              # row-tiles per core (50)
NCH = 5                    # DMA/compute chunks per core
TPCH = NT // NCH           # tiles per chunk (10)
CHW = TPCH * P             # rows per chunk (1280)
NPAD = NCORES * MPC

# Module-level switches (test.py pokes these; harness uses defaults)
TRACE = False
TMPDIR = None
# "bf16": x and z in bfloat16 (halves HBM traffic; rel err ~6e-3)
# "f32": x as float32r, z as float32 (rel err ~1.5e-4)
VARIANT = "bf16"

_cache = {}


def _build_nc(variant):
    f32 = mybir.dt.float32
    if variant == "bf16":
        in_dt, out_dt = mybir.dt.bfloat16, mybir.dt.bfloat16
    else:
        in_dt, out_dt = mybir.dt.float32r, mybir.dt.float32

    nc = bacc.Bacc()
    xtd = nc.dram_tensor("xt", [NCH * P, 2 * CHW], in_dt, kind="ExternalInput")
    snd = nc.dram_tensor("sn", [P, NT], f32, kind="ExternalInput")
    wd = nc.dram_tensor("w", [2 * C, CO], f32, kind="ExternalInput")
    outd = nc.dram_tensor("out", [NCH * P, TPCH * CO], out_dt,
                          kind="ExternalOutput")

    with tile.TileContext(nc) as tc, ExitStack() as ctx:
        const = ctx.enter_context(tc.tile_pool(name="const", bufs=1))
        xp = ctx.enter_context(tc.tile_pool(name="x", bufs=4))
        obp = ctx.enter_context(tc.tile_pool(name="ob", bufs=3))
        psp = ctx.enter_context(tc.tile_pool(name="ps", bufs=8, space="PSUM"))

        # --- one-time setup (ACT ring, so chunk 0's in-DMA leads the SP ring)
        wt = const.tile([P, 4, CO], f32)
        nc.scalar.dma_start(wt[:], wd.rearrange("(q p) n -> p q n", p=P))
        sn = const.tile([P, NT], f32)
        nc.scalar.dma_start(sn[:], snd[:])
        # fold: w_eff chunk q = w[q*128:+128] + w[256 + q*128:+128]
        # (DVE output-casts to the matmul dtype)
        we = const.tile([P, 2, CO], in_dt)
        nc.vector.tensor_add(we[:, 0, :], wt[:, 0, :], wt[:, 2, :])
        nc.vector.tensor_add(we[:, 1, :], wt[:, 1, :], wt[:, 3, :])

        # --- main loop ---
        for ch in range(NCH):
            xc = xp.tile([P, 2 * CHW], in_dt)
            nc.sync.dma_start(xc[:], xtd[ch * P:(ch + 1) * P, :])
            ob = obp.tile([P, TPCH * CO], out_dt)
            for g in range(TPCH):
                t = ch * TPCH + g
                ps = psp.tile([P, CO], f32)
                nc.tensor.matmul(ps[:], xc[:, g * P:g * P + P],
                                 we[:, 0, :], start=True, stop=False)
                nc.tensor.matmul(ps[:], xc[:, CHW + g * P:CHW + g * P + P],
                                 we[:, 1, :], start=False, stop=True)
                # drain with fused symm_norm scale: z = s * (x @ w_eff)
                # (b == 0 for this problem)
                sc = sn[:, t:t + 1]
                if g % 2 == 0:
                    nc.vector.tensor_scalar_mul(ob[:, g * CO:(g + 1) * CO],
                                                ps[:], sc)
                else:
                    nc.scalar.activation(ob[:, g * CO:(g + 1) * CO], ps[:],
                                         mybir.ActivationFunctionType.Copy,
                                         scale=sc)
            nc.scalar.dma_start(outd[ch * P:(ch + 1) * P, :], ob[:])

    nc.finalize()
    return nc


def kernel(x, symm_norm, domains, w, b):
    x = np.asarray(x, dtype=np.float32)
    symm_norm = np.asarray(symm_norm, dtype=np.float32)
    domains = np.asarray(domains)
    w = np.asarray(w, dtype=np.float32)
    b = np.asarray(b, dtype=np.float32)
    assert np.all(b == 0.0), "kernel built for b == 0 (reference uses zeros)"

    in_np = ml_dtypes.bfloat16 if VARIANT == "bf16" else np.float32

    # --- marshal inputs (layout only): pad rows, per-core transposed tiling
    xpad = np.zeros((NPAD, C), dtype=np.float32)
    xpad[:N] = x
    spad = np.zeros(NPAD, dtype=np.float32)
    spad[:N] = symm_norm

    in_maps = []
    for c in range(NCORES):
        R = xpad[c * MPC:(c + 1) * MPC]
        # xt[ch*P + p, q*CHW + m] = R[ch*CHW + m, q*128 + p]
        xt = np.ascontiguousarray(
            R.reshape(NCH, CHW, 2, P).transpose(0, 3, 2, 1)
        ).reshape(NCH * P, 2 * CHW).astype(in_np)
        sc = np.ascontiguousarray(
            spad[c * MPC:(c + 1) * MPC].reshape(NT, P).T)
        in_maps.append({"xt": xt, "sn": sc, "w": w})

    if _cache.get("key") != VARIANT:
        _cache["nc"] = _build_nc(VARIANT)
        _cache["key"] = VARIANT
    nc = _cache["nc"]

    res = run_bass_kernel_spmd(
        nc, in_maps, core_ids=list(range(NCORES)),
        trace=TRACE, tmpdir=TMPDIR,
    )
    _cache["last_results"] = res

    # --- unshard: out[ch*P + p, g*CO + n] = z[ch*CHW + g*128 + p, n]
    zs = []
    for r in res.results:
        o = np.asarray(r["out"]).reshape(NCH, P, TPCH, CO)
        zs.append(o.transpose(0, 2, 1, 3).reshape(MPC, CO))
    z = np.concatenate(zs, axis=0)[:N].astype(np.float32)

    # fan-out: every output row is a copy of one z row (result movement)
    y = np.take(z, domains.reshape(-1), axis=0)
    return y.reshape(D, K, CO)


# revision 3
# speedup vs baseline: 6.5692x; 1.0129x over previous
"""Trainium2 Bass kernel for GNN message-passing conv layer.

Reference computation:
    xs = x * symm_norm[:, None]            # [N, C]
    g  = xs[domains]                        # [D, K, C]
    f  = concat([g, g], -1)                 # [D, K, 2C]
    y  = f @ w + b                          # [D, K, CO]

Algebraic rewrites:
    concat([g, g]) @ w == g @ (w[:C] + w[C:])      (fold doubled channels)
    gather commutes with the row-wise linear map:
        y[d,k] = z[domains[d,k]]  where  z = (x * s) @ w_eff
    (s*x) @ w == s * (x @ w)                       (scale fused into PSUM drain)

So the device computes z for all N nodes once -- a dense [N, C] @ [C, CO]
GEMM row-sharded across the 8 cores (6400 rows each, zero-padded from
50000) -- and the host fans results out with z[domains] (pure result
movement, the same unshard step the gather formulation needs).

Per-core device pipeline, chunked by SCHED (row-tiles of 128 per chunk;
small first chunk starts compute early, small last chunk shrinks the
out-DMA tail):
    in-DMA  (SP ring):  w [128,1024] + sn, then per chunk an xT slice
                        [128, 2*cw] (host-marshalled transposed layout:
                        element [p, q*cw+m] = x[row m, chan q*128+p])
    per tile: 2 accumulating matmuls (k-chunks of 128) into a PSUM tile,
              drain = scale by symm_norm[row] (alternating DVE / ACT)
    out-DMA (ACT ring): z chunk [128, cw*256/128]
"""

import numpy as np
import ml_dtypes
from contextlib import ExitStack

import concourse.bass as bass
import concourse.bacc as bacc
import concourse.mybir as mybir
import concourse.tile as tile
from concourse.bass_utils import run_bass_kernel_spmd

# Problem shapes (hardcoded per contract)
N, C, D, K, CO = 50000, 256, 25000, 16, 256
NCORES = 8
P = 128
MPC = 6400                 # rows per core (8*6400 = 51200 >= N, zero-padded)
NT = MPC // P              # row-tiles per core (50)
SCHED = [2, 8, 10, 10, 10, 8, 2]   # tiles per chunk (sum = NT)
assert sum(SCHED) == NT
TCUM = [sum(SCHED[:i]) for i in range(len(SCHED))]  # tile offset per chunk
NPAD = NCORES * MPC

# Module-level switches (test.py pokes these; harness uses defaults)
TRACE = False
TMPDIR = None
# "bf16": x, w, z in bfloat16 (halves HBM traffic; rel err ~3e-3)
# "f32": x, w as float32r, z as float32 (rel err ~1.5e-4)
VARIANT = "bf16"

_cache = {}


def _build_nc(variant):
    f32 = mybir.dt.float32
    if variant == "bf16":
        in_dt, out_dt = mybir.dt.bfloat16, mybir.dt.bfloat16
    else:
        in_dt, out_dt = mybir.dt.float32r, mybir.dt.float32

    nc = bacc.Bacc()
    xtd = nc.dram_tensor("xt", [P, 2 * MPC], in_dt, kind="ExternalInput")
    snd = nc.dram_tensor("sn", [P, NT], f32, kind="ExternalInput")
    # host-marshalled layout: wq[p, q*CO+n] = w[q*128+p, n]
    wqd = nc.dram_tensor("w", [P, 4 * CO], in_dt, kind="ExternalInput")
    outd = nc.dram_tensor("out", [P, NT * CO], out_dt, kind="ExternalOutput")

    with tile.TileContext(nc) as tc, ExitStack() as ctx:
        const = ctx.enter_context(tc.tile_pool(name="const", bufs=1))
        xp = ctx.enter_context(tc.tile_pool(name="x", bufs=4))
        obp = ctx.enter_context(tc.tile_pool(name="ob", bufs=3))
        psp = ctx.enter_context(tc.tile_pool(name="ps", bufs=8, space="PSUM"))

        # --- one-time setup, at the FRONT of the SP ring (leads chunk 0)
        wt = const.tile([P, 4 * CO], in_dt)
        nc.sync.dma_start(wt[:], wqd[:])
        sn = const.tile([P, NT], f32)
        nc.sync.dma_start(sn[:], snd[:])
        # fold: w_eff chunk q = w[q*128:+128] + w[256 + q*128:+128]
        we = const.tile([P, 2, CO], in_dt)
        nc.vector.tensor_add(we[:, 0, :], wt[:, 0:CO], wt[:, 2 * CO:3 * CO])
        nc.vector.tensor_add(we[:, 1, :], wt[:, CO:2 * CO], wt[:, 3 * CO:])

        # --- main loop ---
        for ch, tcnt in enumerate(SCHED):
            cw = tcnt * P
            xoff = 2 * TCUM[ch] * P
            xc = xp.tile([P, 2 * cw], in_dt)
            nc.sync.dma_start(xc[:], xtd[:, xoff:xoff + 2 * cw])
            ob = obp.tile([P, tcnt * CO], out_dt)
            for g in range(tcnt):
                t = TCUM[ch] + g
                ps = psp.tile([P, CO], f32)
                nc.tensor.matmul(ps[:], xc[:, g * P:g * P + P],
                                 we[:, 0, :], start=True, stop=False)
                nc.tensor.matmul(ps[:], xc[:, cw + g * P:cw + g * P + P],
                                 we[:, 1, :], start=False, stop=True)
                # drain with fused symm_norm scale: z = s * (x @ w_eff)
                # (b == 0 for this problem)
                sc = sn[:, t:t + 1]
                if t % 2 == 0:
                    nc.vector.tensor_scalar_mul(ob[:, g * CO:(g + 1) * CO],
                                                ps[:], sc)
                else:
                    nc.scalar.activation(ob[:, g * CO:(g + 1) * CO], ps[:],
                                         mybir.ActivationFunctionType.Copy,
                                         scale=sc)
            ooff = TCUM[ch] * CO
            nc.scalar.dma_start(outd[:, ooff:ooff + tcnt * CO], ob[:])

    nc.finalize()
    return nc


def kernel(x, symm_norm, domains, w, b):
    x = np.asarray(x, dtype=np.float32)
    symm_norm = np.asarray(symm_norm, dtype=np.float32)
    domains = np.asarray(domains)
    w = np.asarray(w, dtype=np.float32)
    b = np.asarray(b, dtype=np.float32)
    assert np.all(b == 0.0), "kernel built for b == 0 (reference uses zeros)"

    in_np = ml_dtypes.bfloat16 if VARIANT == "bf16" else np.float32

    # --- marshal inputs (layout only): pad rows, per-core transposed tiling
    xpad = np.zeros((NPAD, C), dtype=np.float32)
    xpad[:N] = x
    spad = np.zeros(NPAD, dtype=np.float32)
    spad[:N] = symm_norm
    # wq[p, q*CO+n] = w[q*128+p, n]
    wq = np.ascontiguousarray(
        w.reshape(4, P, CO).transpose(1, 0, 2).reshape(P, 4 * CO)
    ).astype(in_np)

    in_maps = []
    for c in range(NCORES):
        R = xpad[c * MPC:(c + 1) * MPC]
        # per chunk: xt[:, 2*tcum*P + q*cw + m] = R[tcum*P + m, q*128 + p]
        blocks = [
            R[TCUM[ch] * P:(TCUM[ch] + tc) * P]
            .reshape(tc * P, 2, P).transpose(2, 1, 0).reshape(P, 2 * tc * P)
            for ch, tc in enumerate(SCHED)
        ]
        xt = np.ascontiguousarray(np.concatenate(blocks, 1)).astype(in_np)
        sc = np.ascontiguousarray(
            spad[c * MPC:(c + 1) * MPC].reshape(NT, P).T)
        in_maps.append({"xt": xt, "sn": sc, "w": wq})

    if _cache.get("key") != VARIANT:
        _cache["nc"] = _build_nc(VARIANT)
        _cache["key"] = VARIANT
    nc = _cache["nc"]

    res = run_bass_kernel_spmd(
        nc, in_maps, core_ids=list(range(NCORES)),
        trace=TRACE, tmpdir=TMPDIR,
    )
    _cache["last_results"] = res

    # --- unshard: out[p, (tcum+g)*CO + n] = z[(tcum+g)*128 + p, n]
    zs = []
    for r in res.results:
        o = np.asarray(r["out"])
        zs.append(np.concatenate([
            o[:, TCUM[ch] * CO:(TCUM[ch] + tc) * CO]
            .reshape(P, tc, CO).transpose(1, 0, 2).reshape(tc * P, CO)
            for ch, tc in enumerate(SCHED)
        ], 0))
    z = np.concatenate(zs, axis=0).reshape(NCORES * MPC, CO)[:N]
    z = z.astype(np.float32)

    # fan-out: every output row is a copy of one z row (result movement)
    y = np.take(z, domains.reshape(-1), axis=0)
    return y.reshape(D, K, CO)


# revision 5
# speedup vs baseline: 7.0363x; 1.0711x over previous
"""Trainium2 Bass kernel for GNN message-passing conv layer.

Reference computation:
    xs = x * symm_norm[:, None]            # [N, C]
    g  = xs[domains]                        # [D, K, C]
    f  = concat([g, g], -1)                 # [D, K, 2C]
    y  = f @ w + b                          # [D, K, CO]

Algebraic rewrites:
    concat([g, g]) @ w == g @ (w[:C] + w[C:])      (fold doubled channels)
    gather commutes with the row-wise linear map:
        y[d,k] = z[domains[d,k]]  where  z = (x * s) @ w_eff
    (s*x) @ w == s * (x @ w)                       (scale fused into PSUM drain)

So the device computes z for all N nodes once -- a dense [N, C] @ [C, CO]
GEMM row-sharded across the 8 cores (6400 rows each, zero-padded from
50000) -- and the host fans results out with z[domains] (pure result
movement, the same unshard step the gather formulation needs).

Per-core device pipeline, chunked by SCHED (row-tiles of 128 per chunk;
small first chunk starts compute early, small last chunk shrinks the
out-DMA tail):
    in-DMA  (SP ring):  w [128,1024] + sn, then per chunk an xT slice
                        [128, 2*cw] (host-marshalled transposed layout:
                        element [p, q*cw+m] = x[row m, chan q*128+p])
    per tile: 2 accumulating matmuls (k-chunks of 128) into a PSUM tile,
              drain = scale by symm_norm[row] (alternating DVE / ACT)
    out-DMA (ACT ring): z chunk [128, cw*256/128]
"""

import numpy as np
import ml_dtypes
from contextlib import ExitStack

import concourse.bass as bass
import concourse.bacc as bacc
import concourse.mybir as mybir
import concourse.tile as tile
from concourse.bass_utils import run_bass_kernel_spmd

# Problem shapes (hardcoded per contract)
N, C, D, K, CO = 50000, 256, 25000, 16, 256
NCORES = 8
P = 128
MPC = 6400                 # rows per core (8*6400 = 51200 >= N, zero-padded)
NT = MPC // P              # row-tiles per core (50)
SCHED = [2, 8, 10, 10, 10, 8, 2]   # tiles per chunk (sum = NT)
assert sum(SCHED) == NT
TCUM = [sum(SCHED[:i]) for i in range(len(SCHED))]  # tile offset per chunk
NPAD = NCORES * MPC

# Module-level switches (test.py pokes these; harness uses defaults)
TRACE = False
TMPDIR = None
# "bf16": x, w, z in bfloat16 (halves HBM traffic; rel err ~3e-3)
# "f32": x, w as float32r, z as float32 (rel err ~1.5e-4)
VARIANT = "bf16"

_cache = {}


def _build_nc(variant):
    f32 = mybir.dt.float32
    if variant == "bf16":
        in_dt, out_dt = mybir.dt.bfloat16, mybir.dt.bfloat16
    else:
        in_dt, out_dt = mybir.dt.float32r, mybir.dt.float32

    nc = bacc.Bacc()
    xtd = nc.dram_tensor("xt", [P, 2 * MPC], in_dt, kind="ExternalInput")
    snd = nc.dram_tensor("sn", [P, NT], f32, kind="ExternalInput")
    # host-marshalled layout: wq[p, q*CO+n] = w[q*128+p, n]
    wqd = nc.dram_tensor("w", [P, 4 * CO], in_dt, kind="ExternalInput")
    outd = nc.dram_tensor("out", [P, NT * CO], out_dt, kind="ExternalOutput")

    with tile.TileContext(nc) as tc, ExitStack() as ctx:
        const = ctx.enter_context(tc.tile_pool(name="const", bufs=1))
        xp = ctx.enter_context(tc.tile_pool(name="x", bufs=len(SCHED)))
        obp = ctx.enter_context(tc.tile_pool(name="ob", bufs=4))
        psp = ctx.enter_context(tc.tile_pool(name="ps", bufs=7, space="PSUM"))
        wup = ctx.enter_context(tc.tile_pool(name="wu", bufs=1, space="PSUM"))

        # --- one-time setup, at the FRONT of the SP ring (leads chunk 0)
        wt = const.tile([P, 4 * CO], in_dt)
        nc.sync.dma_start(wt[:], wqd[:])
        sn = const.tile([P, NT], f32)
        nc.sync.dma_start(sn[:], snd[:])
        # fold: w_eff chunk q = w[q*128:+128] + w[256 + q*128:+128]
        we = const.tile([P, 2, CO], in_dt)
        nc.vector.tensor_add(we[:, 0, :], wt[:, 0:CO], wt[:, 2 * CO:3 * CO])
        nc.vector.tensor_add(we[:, 1, :], wt[:, CO:2 * CO], wt[:, 3 * CO:])

        # PE warm-up burst: ~3.4us of dummy matmuls on a memset scratch while
        # the x DMAs land, so the HAM clock gate releases (1.2 -> 2.4 GHz)
        # before real compute starts. Results are never read.
        warm = const.tile([P, 512], in_dt)
        nc.vector.memset(warm[:], 0.0)
        wps = wup.tile([P, 512], f32)
        for _ in range(8):
            nc.tensor.matmul(wps[:], warm[:, 0:P], warm[:],
                             start=True, stop=True)

        # --- main loop ---
        for ch, tcnt in enumerate(SCHED):
            cw = tcnt * P
            xoff = 2 * TCUM[ch] * P
            xc = xp.tile([P, 2 * cw], in_dt)
            nc.sync.dma_start(xc[:], xtd[:, xoff:xoff + 2 * cw])
            ob = obp.tile([P, tcnt * CO], out_dt)
            for g in range(tcnt):
                t = TCUM[ch] + g
                ps = psp.tile([P, CO], f32)
                nc.tensor.matmul(ps[:], xc[:, g * P:g * P + P],
                                 we[:, 0, :], start=True, stop=False)
                nc.tensor.matmul(ps[:], xc[:, cw + g * P:cw + g * P + P],
                                 we[:, 1, :], start=False, stop=True)
                # drain with fused symm_norm scale: z = s * (x @ w_eff)
                # (b == 0 for this problem)
                sc = sn[:, t:t + 1]
                if t % 2 == 0:
                    nc.vector.tensor_scalar_mul(ob[:, g * CO:(g + 1) * CO],
                                                ps[:], sc)
                else:
                    nc.scalar.activation(ob[:, g * CO:(g + 1) * CO], ps[:],
                                         mybir.ActivationFunctionType.Copy,
                                         scale=sc)
            ooff = TCUM[ch] * CO
            nc.scalar.dma_start(outd[:, ooff:ooff + tcnt * CO], ob[:])

    nc.finalize()
    return nc


def kernel(x, symm_norm, domains, w, b):
    x = np.asarray(x, dtype=np.float32)
    symm_norm = np.asarray(symm_norm, dtype=np.float32)
    domains = np.asarray(domains)
    w = np.asarray(w, dtype=np.float32)
    b = np.asarray(b, dtype=np.float32)
    assert np.all(b == 0.0), "kernel built for b == 0 (reference uses zeros)"

    in_np = ml_dtypes.bfloat16 if VARIANT == "bf16" else np.float32

    # --- marshal inputs (layout only): pad rows, per-core transposed tiling
    xpad = np.zeros((NPAD, C), dtype=np.float32)
    xpad[:N] = x
    spad = np.zeros(NPAD, dtype=np.float32)
    spad[:N] = symm_norm
    # wq[p, q*CO+n] = w[q*128+p, n]
    wq = np.ascontiguousarray(
        w.reshape(4, P, CO).transpose(1, 0, 2).reshape(P, 4 * CO)
    ).astype(in_np)

    in_maps = []
    for c in range(NCORES):
        R = xpad[c * MPC:(c + 1) * MPC]
        # per chunk: xt[:, 2*tcum*P + q*cw + m] = R[tcum*P + m, q*128 + p]
        blocks = [
            R[TCUM[ch] * P:(TCUM[ch] + tc) * P]
            .reshape(tc * P, 2, P).transpose(2, 1, 0).reshape(P, 2 * tc * P)
            for ch, tc in enumerate(SCHED)
        ]
        xt = np.ascontiguousarray(np.concatenate(blocks, 1)).astype(in_np)
        sc = np.ascontiguousarray(
            spad[c * MPC:(c + 1) * MPC].reshape(NT, P).T)
        in_maps.append({"xt": xt, "sn": sc, "w": wq})

    if _cache.get("key") != VARIANT:
        _cache["nc"] = _build_nc(VARIANT)
        _cache["key"] = VARIANT
    nc = _cache["nc"]

    res = run_bass_kernel_spmd(
        nc, in_maps, core_ids=list(range(NCORES)),
        trace=TRACE, tmpdir=TMPDIR,
    )
    _cache["last_results"] = res

    # --- unshard: out[p, (tcum+g)*CO + n] = z[(tcum+g)*128 + p, n]
    zs = []
    for r in res.results:
        o = np.asarray(r["out"])
        zs.append(np.concatenate([
            o[:, TCUM[ch] * CO:(TCUM[ch] + tc) * CO]
            .reshape(P, tc, CO).transpose(1, 0, 2).reshape(tc * P, CO)
            for ch, tc in enumerate(SCHED)
        ], 0))
    z = np.concatenate(zs, axis=0).reshape(NCORES * MPC, CO)[:N]
    z = z.astype(np.float32)

    # fan-out: every output row is a copy of one z row (result movement)
    y = np.take(z, domains.reshape(-1), axis=0)
    return y.reshape(D, K, CO)
